# revision 14
# baseline (speedup 1.0000x reference)
"""Trainium2 Bass kernel for nn_MANO1D (galerkin linear attention, 8 cores).

Algebraic collapse: with no nonlinearity between the projections, the whole
module reduces to  out[b] = queries[b] @ G[b] + bout  with

    Sraw[b] = keys[b]^T @ values[b]                      # [64, 64]
    G[b]    = sum_h U_h @ Sraw[b] @ Z_h                  # [64, 64]
    U_h     = Wq_h^T @ Wk_h                              # host precomputed
    Z_h     = (Wout_h @ Wk_h)^T / N                      # host precomputed

Sharding: core c handles (batch b = c//2, half = c%2 of the sequence).
Each core computes a partial Sraw over its 8192 rows of keys/values.  The
two cores of a pair then AllReduce their 64x64 partial S (16KB, replica
groups [[0,1],[2,3],[4,5],[6,7]]), so each core has the full-batch S and
computes the FINAL output for only its own half of the sequence.  This
halves the queries load and the output store vs. duplicating them.

Pipelining: kv is loaded in 8 chunked dma_starts alternating between the
SP (sync) and gpsimd HWDGE rings so the phase-1 matmul chain starts after
~1/8 of kv instead of after the whole tensor; qt streams behind kv and
phase-2 consumes it tile by tile with stores trailing the psum copies.
The scalar (ACT) ring only carries phase-2 copies + stores, so its
startup ACT_TABLE_LOAD does not delay the load streams.

Device layouts (128-partition for full DMA bandwidth):
  kv  [128, 8192]  chunk-major: kv[p, 128c+e] = concat(K,V)[128c+p, e]
  qt  [128, 4096]  rows 0:64 = Q^T[:, q0:q0+4096], rows 64:128 = next 4096
  w   [64, 1024]   = [UT_pack | Z_cat]  (weights, shared by all cores)
  ot  [128, 4096]  output, same packing as qt
"""

import ml_dtypes
import numpy as np

import concourse.bacc as bacc
import concourse.mybir as mybir
import concourse.tile as tile
from concourse.bass_utils import run_bass_kernel_spmd

B, N, D, H = 4, 16384, 64, 8
HALF = N // 2            # 8192 rows of k/v per core
CH = HALF // 128         # 64 contraction chunks for Sraw

USE_CC = False           # pair AllReduce of partial S; qt/ot halved.
                         # Measured: the 16KB AllReduce costs ~40us of
                         # latency under this runtime — far more than the
                         # 2MB of DMA it saves.  Keep off.
QC = HALF // 2 if USE_CC else HALF   # qt/ot free size per core

_cached = None


def _build():
    global _cached
    if _cached is not None:
        return _cached

    f32 = mybir.dt.float32
    f32r = mybir.dt.float32r
    bf16 = mybir.dt.bfloat16
    kdt = bf16
    qdt = bf16
    odt = bf16

    nc = bacc.Bacc("TRN2", debug=False, num_devices=8, enable_asserts=False)
    # Drop the constructor preamble we don't use: the four const-AP memsets
    # (nothing reads them here) and the entry all-engine butterfly (~2.9 us on
    # HW).  Body ordering is fully covered by Tile-generated semaphores, and
    # NRT zero-initializes semaphores at load.
    _entry = nc.m.functions[0].blocks[0]
    _entry.instructions[:] = [
        i
        for i in _entry.instructions
        if not (
            str(getattr(i, "opcode", "")).endswith(("Memset", "Drain"))
            or str(i.name).startswith("barrier_")
        )
    ]
    kv_ap = nc.dram_tensor("kv", [128, CH * 128], kdt, kind="ExternalInput").ap()
    qt_ap = nc.dram_tensor("qt", [128, QC], qdt, kind="ExternalInput").ap()
    w_ap = nc.dram_tensor("w", [64, 1024], bf16, kind="ExternalInput").ap()
    ot_ap = nc.dram_tensor("ot", [128, QC], odt, kind="ExternalOutput").ap()

    with tile.TileContext(nc) as tc:
        with (
            tc.tile_pool(name="data", bufs=1) as data,
            tc.tile_pool(name="small", bufs=1) as small,
            tc.tile_pool(name="dram", bufs=2, space="DRAM") as dram,
            tc.tile_pool(name="ps", bufs=1, space="PSUM") as ps,
            tc.tile_pool(name="psout", bufs=6, space="PSUM") as psout,
        ):
            kv_sb = data.tile([128, CH * 128], kdt)
            qt_sb = data.tile([128, QC], qdt)
            ot_sb = data.tile([128, QC], odt)
            # (g_sb removed: phase 2 uses the blockdiag g2_sb instead)
            w_sb = small.tile([64, 1024], bf16)
            st_sb = small.tile([64, 64], f32 if USE_CC else bf16)
            g2_sb = small.tile([128, 128], qdt)
            y_sb = small.tile([64, 512], bf16)
            if USE_CC:
                s2_sb = small.tile([64, 64], f32)
                s2r_sb = small.tile([64, 64], bf16)
                s_in = dram.tile([64, 64], f32)
                s_out = dram.tile([64, 64], f32)

            # Loads.  Only sync (SP) and scalar (ACT) have HWDGE rings —
            # descriptor generation is a SEQUENCER instruction, so it does
            # not queue behind scalar-engine compute.  kv goes first in 8
            # chunked dma_starts alternating rings so phase-1 matmuls
            # pipeline behind the stream (the per-queue FIFOs then drain kv
            # descriptors before qt's); then w, then qt.
            # kv in 8 partition-split dma_starts: [64p x 2048c] each = 4KB
            # descriptors, 64 per start.  Dense generation enqueues ALL kv
            # descriptors within ~3us, so the later qt descriptors (ring
            # credit) cannot steal queue slots from the kv stream.  Col
            # group g needs both partition halves: p0 from sync, p1 from
            # scalar, generated in lockstep.
            kv_grp = 2048
            for g in range(CH * 128 // kv_grp):
                c0 = g * kv_grp
                nc.sync.dma_start(kv_sb[0:64, c0 : c0 + kv_grp], kv_ap[0:64, c0 : c0 + kv_grp])
                if g == 0:
                    nc.scalar.dma_start(
                        kv_sb[64:128, c0 : c0 + kv_grp], kv_ap[64:128, c0 : c0 + kv_grp]
                    )
                    nc.scalar.dma_start(w_sb[:], w_ap[:])
                else:
                    nc.scalar.dma_start(
                        kv_sb[64:128, c0 : c0 + kv_grp], kv_ap[64:128, c0 : c0 + kv_grp]
                    )
            qt_grp = 2048
            for g in range(QC // qt_grp):
                ring = nc.sync if g % 2 == 0 else nc.scalar
                c0 = g * qt_grp
                ring.dma_start(qt_sb[:, c0 : c0 + qt_grp], qt_ap[:, c0 : c0 + qt_grp])

            nc.gpsimd.memset(g2_sb[:], 0.0)

            # Phase 1: Sraw^T = V^T K over 64 chunks of 128 rows.  Even/odd
            # chunks accumulate into disjoint psum partition halves so
            # consecutive matmuls overlap on the PE.
            ps_st = ps.tile([128, 64], f32, tag="sm", bufs=2)
            for c in range(CH):
                p0 = 64 * (c % 2)
                nc.tensor.matmul(
                    ps_st[p0 : p0 + 64, :],
                    lhsT=kv_sb[:, c * 128 + 64 : c * 128 + 128],
                    rhs=kv_sb[:, c * 128 : c * 128 + 64],
                    start=(c < 2),
                    stop=(c >= CH - 2),
                )
            nc.vector.tensor_copy(st_sb[:], ps_st[0:64, :])
            nc.vector.tensor_add(st_sb[:], st_sb[:], ps_st[64:128, :])

            if USE_CC:
                # Pairwise sum of the partial S through a DRAM bounce:
                # 16KB AllReduce between the two cores holding one batch.
                nc.gpsimd.dma_start(s_in[:], st_sb[:])
                nc.gpsimd.collective_compute(
                    "AllReduce",
                    mybir.AluOpType.add,
                    replica_groups=[[0, 1], [2, 3], [4, 5], [6, 7]],
                    ins=[s_in.opt()],
                    outs=[s_out.opt()],
                )
                nc.gpsimd.dma_start(s2_sb[:], s_out[:])
                nc.vector.tensor_copy(s2r_sb[:], s2_sb[:])
                s_for_y = s2r_sb
            else:
                s_for_y = st_sb

            # Y_cat = S @ Z_cat, split in two halves so the second matmul
            # overlaps the first half's psum->sbuf cast.
            for yy in (0, 256):
                ps_y = ps.tile([64, 256], f32, tag="sm", bufs=2, name=f"ps_y{yy}")
                nc.tensor.matmul(
                    ps_y[:],
                    lhsT=s_for_y[:],
                    rhs=w_sb[:, 512 + yy : 768 + yy],
                    start=True,
                    stop=True,
                )
                nc.vector.tensor_copy(y_sb[:, yy : yy + 256], ps_y[:])

            # G = sum_h U_h @ Y_h   (8 accumulating matmuls)
            ps_g = ps.tile([64, 64], f32, tag="sm", bufs=2)
            for h in range(H):
                nc.tensor.matmul(
                    ps_g[:],
                    lhsT=w_sb[:, 64 * h : 64 * h + 64],
                    rhs=y_sb[:, 64 * h : 64 * h + 64],
                    start=(h == 0),
                    stop=(h == H - 1),
                )
            # blockdiag(G, G) [128, 128]: one 128-partition stationary lets a
            # single matmul per 512-col tile produce BOTH sequence-quarter
            # outputs (upper partitions contract against G, lower against G,
            # zero off-diagonal blocks kill the cross terms).  Halves the
            # phase-2 moving-row count vs two 64-partition quadrant matmuls.
            nc.vector.tensor_copy(g2_sb[0:64, 0:64], ps_g[:])
            nc.vector.tensor_copy(g2_sb[64:128, 64:128], ps_g[:])

            # Phase 2: out^T = G^T @ Q^T.  The two sequence quarters sit on
            # partition ranges 0:64 / 64:128: their matmuls land in disjoint
            # (row, col) quadrants of the PE array and disjoint partition
            # halves of ONE psum bank, so they run concurrently and a single
            # [128, 512] copy drains both.  Stores trail the copies so the
            # output stream overlaps the tail of phase 2.
            NT2 = QC // 512
            store_plan = {
                1: (0, 1024, nc.sync),
                3: (1024, 2048, nc.scalar),
                5: (2048, 3072, nc.sync),
                6: (3072, 3584, nc.scalar),
                7: (3584, 4096, nc.sync),
            }
            if not USE_CC:
                store_plan = {
                    1: (0, 1024, nc.sync),
                    3: (1024, 2048, nc.scalar),
                    5: (2048, 3072, nc.sync),
                    7: (3072, 4096, nc.scalar),
                    9: (4096, 5120, nc.sync),
                    11: (5120, 6144, nc.scalar),
                    13: (6144, 7168, nc.sync),
                    14: (7168, 7680, nc.scalar),
                    15: (7680, 8192, nc.sync),
                }
            for t in range(NT2):
                c0 = t * 512
                po = psout.tile([128, 512], f32)
                nc.tensor.matmul(
                    po[:],
                    lhsT=g2_sb[:],
                    rhs=qt_sb[:, c0 : c0 + 512],
                    start=True,
                    stop=True,
                )
                copy = nc.vector.tensor_copy if t % 2 == 0 else nc.scalar.copy
                copy(ot_sb[:, c0 : c0 + 512], po[:])
                if t in store_plan:
                    s0, s1, ring = store_plan[t]
                    ring.dma_start(ot_ap[:, s0:s1], ot_sb[:, s0:s1])

    # Tail surgery: Tile's epilogue is [store-completion drain, barrier #1,
    # semaphore range-clear, barrier #2].  Barrier #2 only makes every engine
    # wait for the clear; NEFF completion already requires each engine stream
    # (clear included) to finish, so drop everything after the clear.
    for bb in nc.m.functions[0].blocks:
        if bb.name.endswith("_end"):
            insts = bb.instructions
            isa_idx = [
                i
                for i, x in enumerate(insts)
                if str(getattr(x, "opcode", "")).endswith("ISA")
            ]
            if isa_idx:
                del insts[isa_idx[-1] + 1 :]

    nc.compile()
    _cached = nc
    return nc


def kernel(queries, keys, values, Wq, Wk, Wout, bout):
    queries = np.asarray(queries, np.float32)
    keys = np.asarray(keys, np.float32)
    values = np.asarray(values, np.float32)
    Wq = np.asarray(Wq, np.float32)
    Wk = np.asarray(Wk, np.float32)
    Wout = np.asarray(Wout, np.float32)
    bout = np.asarray(bout, np.float32)

    nc = _build()

    # Host precompute of the folded weight matrices (tiny).
    UT_pack = np.empty((64, 512), np.float32)
    Z_cat = np.empty((64, 512), np.float32)
    for h in range(H):
        Wq_h = Wq[64 * h : 64 * h + 64, :]
        Wk_h = Wk[64 * h : 64 * h + 64, :]
        Wout_h = Wout[:, 64 * h : 64 * h + 64]
        UT_pack[:, 64 * h : 64 * h + 64] = (Wq_h.T @ Wk_h).T
        Z_cat[:, 64 * h : 64 * h + 64] = (Wout_h @ Wk_h).T / np.float32(N)
    w_in = np.ascontiguousarray(
        np.concatenate([UT_pack, Z_cat], axis=1)
    ).astype(ml_dtypes.bfloat16)

    in_maps = []
    for c in range(8):
        b, half = c // 2, c % 2
        r0 = half * HALF
        kv_rows = np.concatenate(
            [keys[b, r0 : r0 + HALF], values[b, r0 : r0 + HALF]], axis=1
        )  # [8192, 128]
        kv = np.ascontiguousarray(
            kv_rows.reshape(CH, 128, 128).transpose(1, 0, 2).reshape(128, CH * 128)
        ).astype(ml_dtypes.bfloat16)
        qT = queries[b].T  # [64, 16384]
        if USE_CC:
            seg = qT[:, r0 : r0 + HALF]  # this core's half only
            qtp = np.ascontiguousarray(
                np.concatenate([seg[:, :QC], seg[:, QC:]], axis=0)
            ).astype(ml_dtypes.bfloat16)
        else:
            qtp = np.ascontiguousarray(
                np.concatenate([qT[:, :HALF], qT[:, HALF:]], axis=0)
            ).astype(ml_dtypes.bfloat16)
        in_maps.append({"kv": kv, "qt": qtp, "w": w_in})

    res = run_bass_kernel_spmd(nc, in_maps, core_ids=list(range(8)))

    out = np.empty((B, N, D), np.float32)
    if USE_CC:
        for c in range(8):
            b, half = c // 2, c % 2
            s = res.results[c]["ot"].astype(np.float32)  # [128, 4096]
            seg = np.concatenate([s[0:64], s[64:128]], axis=1)  # [64, 8192]
            out[b, half * HALF : (half + 1) * HALF] = seg.T
        out += bout
    else:
        for b in range(B):
            s = res.results[2 * b]["ot"].astype(np.float32) + res.results[
                2 * b + 1
            ]["ot"].astype(np.float32)  # [128, 8192]
            outT = np.concatenate([s[0:64], s[64:128]], axis=1)  # [64, 16384]
            out[b] = outT.T + bout
    return out


# revision 15
# speedup vs baseline: 1.4329x; 1.4329x over previous
"""Trainium2 Bass kernel for nn_MANO1D (galerkin linear attention, 8 cores).

Algebraic collapse: with no nonlinearity between the projections, the whole
module reduces to  out[b] = queries[b] @ G[b] + bout  with

    Sraw[b] = keys[b]^T @ values[b]                      # [64, 64]
    G[b]    = sum_h U_h @ Sraw[b] @ Z_h                  # [64, 64]
    U_h     = Wq_h^T @ Wk_h                              # host precomputed
    Z_h     = (Wout_h @ Wk_h)^T / N                      # host precomputed

Sharding: core c handles (batch b = c//2, half = c%2 of the sequence).
Each core computes a partial Sraw over its 8192 rows of keys/values, the
(linear-in-S) partial G, then the partial output  queries[b] @ G_partial
over the full sequence.  The host sums the two partials per batch and adds
bout.  No cross-core communication needed.

Device layouts (everything 128-partition for full DMA bandwidth):
  kv  [128, 8192]  chunk-major: kv[p, 128c+e] = concat(K,V)[128c+p, e]
  qt  [128, 8192]  rows 0:64 = Q^T[:, :8192], rows 64:128 = Q^T[:, 8192:]
  w   [64, 1024]   = [UT_pack | Z_cat]  (weights, shared by all cores)
  ot  [128, 8192]  output, same packing as qt
"""

import ml_dtypes
import numpy as np

import concourse.bacc as bacc
import concourse.mybir as mybir
import concourse.tile as tile
from concourse.bass_utils import run_bass_kernel_spmd

B, N, D, H = 4, 16384, 64, 8
HALF = N // 2            # 8192 rows of k/v per core; qt/ot free size
CH = HALF // 128         # 64 contraction chunks for Sraw
NT = HALF // 512         # 16 output column tiles per half

KV_BF16 = True           # keys/values input in bf16 (halves phase-1 DMA)
QT_BF16 = True           # Q^T input in bf16 (halves phase-2 input DMA)
OT_BF16 = True           # output in bf16 (halves store DMA)
F32R_PHASE2 = True       # float32r for Q@G when inputs stay fp32

_cached = None


def _build():
    global _cached
    if _cached is not None:
        return _cached

    f32 = mybir.dt.float32
    f32r = mybir.dt.float32r
    bf16 = mybir.dt.bfloat16
    kdt = bf16 if KV_BF16 else f32
    qdt = bf16 if QT_BF16 else (f32r if F32R_PHASE2 else f32)
    odt = bf16 if OT_BF16 else f32
    # HWDGE descriptor generation runs ~18 ns/descriptor and every dma_start
    # emits one descriptor per partition (128), so big per-partition runs are
    # what keep the 16 SDMA engines fed: 12-16KB runs for loads.
    qt_grp = 2048                     # qt columns per dma_start (4KB runs)
    ot_grp = 2048                     # ot columns per dma_start (4KB runs)

    nc = bacc.Bacc("TRN2", debug=False, num_devices=8, enable_asserts=False)
    # Drop the constructor preamble we don't use: the four const-AP memsets
    # (nothing reads them here) and the entry all-engine butterfly (~2.9 us on
    # HW).  Body ordering is fully covered by Tile-generated semaphores, and
    # NRT zero-initializes semaphores at load.
    _entry = nc.m.functions[0].blocks[0]
    _entry.instructions[:] = [
        i
        for i in _entry.instructions
        if not (
            str(getattr(i, "opcode", "")).endswith(("Memset", "Drain"))
            or str(i.name).startswith("barrier_")
        )
    ]
    kv_ap = nc.dram_tensor("kv", [128, CH * 128], kdt, kind="ExternalInput").ap()
    qt_ap = nc.dram_tensor("qt", [128, HALF], qdt, kind="ExternalInput").ap()
    w_ap = nc.dram_tensor("w", [64, 1024], f32r, kind="ExternalInput").ap()
    ot_ap = nc.dram_tensor("ot", [128, HALF], odt, kind="ExternalOutput").ap()

    with tile.TileContext(nc) as tc:
        with (
            tc.tile_pool(name="data", bufs=1) as data,
            tc.tile_pool(name="small", bufs=1) as small,
            tc.tile_pool(name="ps", bufs=1, space="PSUM") as ps,
            tc.tile_pool(name="psout", bufs=6, space="PSUM") as psout,
        ):
            kv_sb = data.tile([128, CH * 128], kdt)
            qt_sb = data.tile([128, HALF], qdt)
            ot_sb = data.tile([128, HALF], odt)
            w_sb = small.tile([64, 1024], f32r)
            st_sb = small.tile([64, 64], f32r)
            y_sb = small.tile([64, 512], f32r)
            g_sb = small.tile([128, 64], qdt)

            # Split every transfer across BOTH HWDGE rings (SP + ACT): the SDMA
            # engines round-robin between rings at packet granularity, so one
            # ring's descriptor-fetch stalls are filled by the other ring.
            # kv absolutely first on BOTH rings (it gates the compute chain),
            # then w, then qt halves, then stores.  The ACT ring streams about
            # half as fast as the SP ring, so split kv ~2:1.
            half_kv = 45 * 128
            nc.sync.dma_start(kv_sb[:, :half_kv], kv_ap[:, :half_kv])
            nc.scalar.dma_start(kv_sb[:, half_kv:], kv_ap[:, half_kv:])
            nc.scalar.dma_start(w_sb[:], w_ap[:])
            for g in range(HALF // qt_grp):
                c0 = g * qt_grp
                ring = nc.sync if g % 2 == 0 else nc.scalar
                ring.dma_start(qt_sb[:, c0 : c0 + qt_grp], qt_ap[:, c0 : c0 + qt_grp])

            # Phase 1: Sraw^T = V^T K over 64 chunks of 128 rows.  Even/odd
            # chunks accumulate into disjoint psum partition halves (col
            # tiling) so consecutive matmuls overlap on the PE.
            # st/y/g psum tiles share one pool slot (sequential lifetimes) so
            # six banks stay free for the phase-2 pipeline.
            ps_st = ps.tile([128, 64], f32, tag="sm", bufs=2)
            for c in range(CH):
                p0 = 64 * (c % 2)
                nc.tensor.matmul(
                    ps_st[p0 : p0 + 64, :],
                    lhsT=kv_sb[:, c * 128 + 64 : c * 128 + 128],
                    rhs=kv_sb[:, c * 128 : c * 128 + 64],
                    start=(c < 2),
                    stop=(c >= CH - 2),
                )
            nc.vector.tensor_copy(st_sb[:], ps_st[0:64, :])
            nc.vector.tensor_add(st_sb[:], st_sb[:], ps_st[64:128, :])

            # Y_cat = Sraw @ Z_cat, split in two halves so the second matmul
            # overlaps the first half's psum->sbuf cast.
            for yy in (0, 256):
                ps_y = ps.tile([64, 256], f32, tag="sm", bufs=2, name=f"ps_y{yy}")
                nc.tensor.matmul(
                    ps_y[:],
                    lhsT=st_sb[:],
                    rhs=w_sb[:, 512 + yy : 768 + yy],
                    start=True,
                    stop=True,
                )
                nc.vector.tensor_copy(y_sb[:, yy : yy + 256], ps_y[:])

            # G = sum_h U_h @ Y_h   (8 accumulating matmuls; heads 0-3 only
            # need the first Y half, so they start under the second half)
            ps_g = ps.tile([64, 64], f32, tag="sm", bufs=2)
            for h in range(H):
                nc.tensor.matmul(
                    ps_g[:],
                    lhsT=w_sb[:, 64 * h : 64 * h + 64],
                    rhs=y_sb[:, 64 * h : 64 * h + 64],
                    start=(h == 0),
                    stop=(h == H - 1),
                )
            nc.vector.tensor_copy(g_sb[0:64, :], ps_g[:])
            nc.vector.tensor_copy(g_sb[64:128, :], ps_g[:])

            # Phase 2: out^T = G^T @ Q^T.  The two sequence halves sit on
            # partition ranges 0:64 / 64:128: their matmuls land in disjoint
            # (row, col) quadrants of the PE array and disjoint partition
            # halves of ONE psum bank, so they run concurrently and a single
            # [128, 512] copy drains both.
            for t in range(NT):
                c0 = t * 512
                po = psout.tile([128, 512], f32)
                for half in (0, 1):
                    p0 = 64 * half
                    nc.tensor.matmul(
                        po[p0 : p0 + 64, :],
                        lhsT=g_sb[p0 : p0 + 64, :],
                        rhs=qt_sb[p0 : p0 + 64, c0 : c0 + 512],
                        start=True,
                        stop=True,
                    )
                copy = nc.vector.tensor_copy if t % 2 == 0 else nc.scalar.copy
                copy(ot_sb[:, c0 : c0 + 512], po[:])

            # Small leading groups so the store stream starts after only two
            # phase-2 copies; small trailing groups on the fast ring so the
            # final store (the exec-time driver) is short.
            for c0, c1, ring in (
                (0, 1024, nc.sync),
                (1024, 2048, nc.scalar),
                (2048, 4096, nc.sync),
                (4096, 6144, nc.scalar),
                (6144, 7168, nc.sync),
                (7168, 8192, nc.sync),
            ):
                ring.dma_start(ot_ap[:, c0:c1], ot_sb[:, c0:c1])

    # Tail surgery: Tile's epilogue is [store-completion drain, barrier #1,
    # semaphore range-clear, barrier #2].  Barrier #2 only makes every engine
    # wait for the clear; NEFF completion already requires each engine stream
    # (clear included) to finish, so drop everything after the clear.
    for bb in nc.m.functions[0].blocks:
        if bb.name.endswith("_end"):
            insts = bb.instructions
            isa_idx = [
                i
                for i, x in enumerate(insts)
                if str(getattr(x, "opcode", "")).endswith("ISA")
            ]
            if isa_idx:
                del insts[isa_idx[-1] + 1 :]

    nc.compile()
    _cached = nc
    return nc


def kernel(queries, keys, values, Wq, Wk, Wout, bout):
    queries = np.asarray(queries, np.float32)
    keys = np.asarray(keys, np.float32)
    values = np.asarray(values, np.float32)
    Wq = np.asarray(Wq, np.float32)
    Wk = np.asarray(Wk, np.float32)
    Wout = np.asarray(Wout, np.float32)
    bout = np.asarray(bout, np.float32)

    nc = _build()

    # Host precompute of the folded weight matrices (tiny).
    UT_pack = np.empty((64, 512), np.float32)
    Z_cat = np.empty((64, 512), np.float32)
    for h in range(H):
        Wq_h = Wq[64 * h : 64 * h + 64, :]
        Wk_h = Wk[64 * h : 64 * h + 64, :]
        Wout_h = Wout[:, 64 * h : 64 * h + 64]
        UT_pack[:, 64 * h : 64 * h + 64] = (Wq_h.T @ Wk_h).T
        Z_cat[:, 64 * h : 64 * h + 64] = (Wout_h @ Wk_h).T / np.float32(N)
    w_in = np.ascontiguousarray(np.concatenate([UT_pack, Z_cat], axis=1))

    kv_np = ml_dtypes.bfloat16 if KV_BF16 else np.float32
    qt_np = ml_dtypes.bfloat16 if QT_BF16 else np.float32

    in_maps = []
    for c in range(8):
        b, half = c // 2, c % 2
        r0 = half * HALF
        kv_rows = np.concatenate(
            [keys[b, r0 : r0 + HALF], values[b, r0 : r0 + HALF]], axis=1
        )  # [8192, 128]
        kv = np.ascontiguousarray(
            kv_rows.reshape(CH, 128, 128).transpose(1, 0, 2).reshape(128, CH * 128)
        ).astype(kv_np)
        qT = queries[b].T  # [64, 16384]
        qtp = np.ascontiguousarray(
            np.concatenate([qT[:, :HALF], qT[:, HALF:]], axis=0)
        ).astype(qt_np)
        in_maps.append({"kv": kv, "qt": qtp, "w": w_in})

    res = run_bass_kernel_spmd(nc, in_maps, core_ids=list(range(8)))

    out = np.empty((B, N, D), np.float32)
    for b in range(B):
        s = res.results[2 * b]["ot"].astype(np.float32) + res.results[2 * b + 1][
            "ot"
        ].astype(np.float32)  # [128, 8192]
        outT = np.concatenate([s[0:64], s[64:128]], axis=1)  # [64, 16384]
        out[b] = outT.T + bout
    return out

